# revision 4
# baseline (speedup 1.0000x reference)
"""Trainium2 Bass kernel for nn_ExplodedLogit (topk_masking).

Reference computation (x (512,256) f32, W (1,256) f32, b (1,) f32):
    scores = x @ W.T + b                                  (512, 1)
    idx    = argmax(scores)
    mask   = ones(512) with log(1e-46) at idx
    block  = scores * mask[None, :]                       (512, 512)
    out    = concat([scores, tile(block, (1, 512))], 1)   (512, 262145)

Sharding: the 512 identical block repetitions are split across 8
NeuronCores, 64 reps each -> per-core "rep" output (512, 32768) = 64 MB
(memory-bound: this is an HBM-write problem). Every core runs the
identical program: scores/argmax/mask are recomputed redundantly (tiny),
and the per-core slice is materialized with fan-out DMAs that read a
small SBUF block through a step-0 (broadcast) access-pattern dim.

Row layout: r = 4p + t (p = partition 0..127, t = 0..3) so the x load is
128 contiguous 4KB descriptors (one DMA).  The mask path avoids the old
4-selector-matmul broadcast: global max runs as DVE reduce + GPSIMD
partition_all_reduce, the per-chunk mask is built on partitions 0..3,
spread block-diagonally (affine_select) and broadcast to all partitions
with ONE single-pass bf16 matmul (mask values are exactly representable
enough: bf16(log 1e-46) = -106.0, a 7.6e-4 relative output error).
"""

import math

import numpy as np

import concourse.bacc as bacc
import concourse.bass_isa as bass_isa
import concourse.bass_utils as _bass_utils
import concourse.mybir as mybir
import concourse.tile as tile
from concourse.bass_utils import run_bass_kernel_spmd

# If profiling is enabled via env (BASS_TRACE), a failed artifact upload
# must not take down the run — fall back to the local tmpdir.
_orig_upload = _bass_utils.upload_artifacts


def _safe_upload(tmpdir):
    try:
        return _orig_upload(tmpdir)
    except Exception:
        return tmpdir


_bass_utils.upload_artifacts = _safe_upload

F32 = mybir.dt.float32
BF16 = mybir.dt.bfloat16
MASK_VAL = float(np.float32(math.log(1e-46)))  # ~ -105.9189

T = 512        # tracks (rows)
F = 256        # features
P = 128        # SBUF partitions
TPP = T // P   # 4 rows per partition (r = 4p + t)
NREP = 512     # total block repetitions in the full output
NCORES = 8
RPC = NREP // NCORES   # 64 reps per core
R = 8                  # reps materialized in SBUF
G = RPC // R           # step-0 groups per fan-out DMA


def _build():
    nc = bacc.Bacc("TRN2", target_bir_lowering=False, debug=False,
                   num_devices=NCORES)
    x = nc.dram_tensor("x", [T, F], F32, kind="ExternalInput")
    W = nc.dram_tensor("W", [1, F], F32, kind="ExternalInput")
    b = nc.dram_tensor("b", [1, 1], F32, kind="ExternalInput")
    rep_out = nc.dram_tensor("rep", [T, RPC * T], F32, kind="ExternalOutput")
    scores_out = nc.dram_tensor("scores", [T, 1], F32, kind="ExternalOutput")

    with tile.TileContext(nc) as tc:
        with (
            tc.tile_pool(name="sbuf", bufs=1) as sbuf_pool,
            tc.tile_pool(name="psum", bufs=1, space="PSUM") as psum_pool,
        ):
            _emit(nc, x[:], W[:], b[:], rep_out[:], scores_out[:],
                  sbuf_pool, psum_pool)
    nc.compile()
    return nc


def _emit(nc, x, W, b, rep_out, scores_out, sbuf_pool, psum_pool):
    x_sb = sbuf_pool.tile([P, TPP * F], F32)     # x[4p+t, f] at [p, t*F+f]
    w_sb = sbuf_pool.tile([P, F], F32)
    b_sb = sbuf_pool.tile([P, 1], F32)
    tmp_v = sbuf_pool.tile([P, 2 * F], F32)      # DVE scratch (chunks 0,1)
    tmp_g = sbuf_pool.tile([P, 2 * F], F32)      # GPSIMD scratch (chunks 2,3)
    sc_sb = sbuf_pool.tile([P, TPP], F32)        # scores: s[4p+t] at [p,t]
    ones_sb = sbuf_pool.tile([P, P], F32)
    id_sb = sbuf_pool.tile([P, P], F32)          # 128x128 identity
    ones4b_sb = sbuf_pool.tile([TPP, P], BF16)   # lhsT for mask broadcast
    mxp_sb = sbuf_pool.tile([P, 8], F32)         # per-partition max (MAX8)
    gmax_sb = sbuf_pool.tile([P, 1], F32)        # global max, all partitions
    ind4_sb = sbuf_pool.tile([TPP, P], F32)      # argmax one-hot, chunked
    mask4b_sb = sbuf_pool.tile([TPP, P], BF16)   # mask values, chunked
    msk4d_sb = sbuf_pool.tile([TPP, P * TPP], BF16)  # block-diag spread
    rep_sb = sbuf_pool.tile([P, TPP * R * T], F32)

    sT_ps = psum_pool.tile([TPP, P], F32)
    mask_ps = psum_pool.tile([P, T], F32)

    # ---- constants (overlap with the x load) ----
    nc.vector.memset(ones_sb[:], 1.0)
    nc.vector.memset(ones4b_sb[:], 1.0)
    # identity: keep ones where (col - row) == 0
    nc.gpsimd.affine_select(
        id_sb[:], ones_sb[:], [[1, P]], mybir.AluOpType.is_equal, 0.0,
        base=0, channel_multiplier=-1,
    )

    # ---- loads (all on the SP ring, x first; scalar keeps its ACT table
    # load off the DMA path) ----
    nc.sync.dma_start(x_sb[:], x.rearrange("(p t) f -> p (t f)", p=P))
    nc.sync.dma_start(w_sb[:], W.broadcast_to((P, F)))
    nc.sync.dma_start(b_sb[:], b.broadcast_to((P, 1)))

    # ---- scores: s[4p+t] = b + sum_f x[4p+t,f] * W[f] ----
    # muls for chunks 2,3 on GPSIMD in parallel with DVE (tensor_tensor
    # never grabs the shared DVE/GPSIMD port pair); reduces are DVE-only
    # (GPSIMD tensor_reduce can't do free-axis)
    for t in (2, 3):
        o = (t - 2) * F
        nc.gpsimd.tensor_mul(
            tmp_g[:, o:o + F], x_sb[:, t * F:(t + 1) * F], w_sb[:]
        )
    for t in (0, 1):
        o = t * F
        nc.vector.tensor_mul(
            tmp_v[:, o:o + F], x_sb[:, t * F:(t + 1) * F], w_sb[:]
        )
        nc.vector.reduce_sum(
            sc_sb[:, t:t + 1], tmp_v[:, o:o + F], axis=mybir.AxisListType.X,
        )
    for t in (2, 3):
        o = (t - 2) * F
        nc.vector.reduce_sum(
            sc_sb[:, t:t + 1], tmp_g[:, o:o + F], axis=mybir.AxisListType.X,
        )
    nc.vector.tensor_scalar_add(sc_sb[:], sc_sb[:], b_sb[:, 0:1])

    # ---- global max: DVE row-max, then GPSIMD all-reduce across partitions
    nc.vector.reduce_max(
        mxp_sb[:, 0:1], sc_sb[:], axis=mybir.AxisListType.X
    )
    nc.gpsimd.partition_all_reduce(
        gmax_sb[:], mxp_sb[:, 0:1], channels=P,
        reduce_op=bass_isa.ReduceOp.max,
    )

    # ---- transpose scores to the free dim (PE): sT[t, p] = s[4p+t] ----
    nc.tensor.matmul(sT_ps[:], lhsT=sc_sb[:], rhs=id_sb[:])
    # external scores output (off the critical path, ACT ring)
    nc.scalar.dma_start(
        scores_out.rearrange("(p t) one -> p (t one)", p=P), sc_sb[:]
    )

    # ---- mask on partitions 0..3: ind = (s == gmax); m = 1 + ind*(MV-1)
    nc.vector.tensor_scalar(
        ind4_sb[:], sT_ps[:], gmax_sb[0:TPP, 0:1], None,
        mybir.AluOpType.is_equal,
    )
    nc.vector.tensor_scalar(
        mask4b_sb[:], ind4_sb[:], MASK_VAL - 1.0, 1.0,
        mybir.AluOpType.mult, mybir.AluOpType.add,
    )
    # block-diagonal spread: msk4d[k, p', t'] = mask4[k, p'] iff t' == k
    # (output column j = 4p' + t' matches row order r = 4p + t)
    nc.gpsimd.affine_select(
        msk4d_sb[:].rearrange("k (m t) -> k m t", t=TPP),
        mask4b_sb[:].unsqueeze(2).broadcast_to((TPP, P, TPP)),
        [[0, P], [1, TPP]], mybir.AluOpType.is_equal, 0.0,
        base=0, channel_multiplier=-1,
    )
    # broadcast to all 128 partitions: ONE single-pass bf16 matmul
    nc.tensor.matmul(mask_ps[:], lhsT=ones4b_sb[:], rhs=msk4d_sb[:])

    # ---- fill rep_sb: R copies of each row's block slice ----
    # rep_sb[p, (t*R+r)*T + c] = sc[p,t] * mask[c]   (mask read from PSUM)
    # t=0 gates the first fan-out DMA: fill its halves on DVE and ACT in
    # parallel and write them with separate DMAs so streaming starts after
    # half a fill. t=1..3 overlap with streaming anyway.
    h = R // 2
    nc.vector.tensor_scalar(
        rep_sb[:, 0:h * T].rearrange("p (r c) -> p r c", c=T),
        mask_ps.unsqueeze(1).broadcast_to((P, h, T)),
        sc_sb[:, 0:1], None, mybir.AluOpType.mult,
    )
    nc.scalar.activation(
        rep_sb[:, h * T:R * T].rearrange("p (r c) -> p r c", c=T),
        mask_ps.unsqueeze(1).broadcast_to((P, h, T)),
        mybir.ActivationFunctionType.Copy,
        scale=sc_sb[:, 0:1],
    )
    for t in range(1, TPP):
        nc.vector.tensor_scalar(
            rep_sb[:, t * R * T:(t + 1) * R * T].rearrange(
                "p (r c) -> p r c", c=T
            ),
            mask_ps.unsqueeze(1).broadcast_to((P, R, T)),
            sc_sb[:, t:t + 1], None, mybir.AluOpType.mult,
        )

    # ---- fan-out DMAs: write each t-slot G times via a step-0 src dim ----
    out_v = rep_out.rearrange("(p t) (g u) -> t p g u", p=P, u=R * T)
    # t=0 in rep-halves so the first write only waits for half a fill
    for half in range(2):
        src = (
            rep_sb[:, half * h * T:(half + 1) * h * T]
            .unsqueeze(1)
            .broadcast_to((P, G, h * T))
        )
        dst = out_v[0][:, :, half * h * T:(half + 1) * h * T]
        nc.sync.dma_start(dst, src)
    for t in range(1, TPP):
        src = (
            rep_sb[:, t * R * T:(t + 1) * R * T]
            .unsqueeze(1)
            .broadcast_to((P, G, R * T))
        )
        nc.sync.dma_start(out_v[t], src)


_NC_CACHE = None


def _get_nc():
    global _NC_CACHE
    if _NC_CACHE is None:
        _NC_CACHE = _build()
    return _NC_CACHE


def _run(x, W, b, **run_kwargs):
    nc = _get_nc()
    in_map = {
        "x": np.ascontiguousarray(np.asarray(x, dtype=np.float32)),
        "W": np.ascontiguousarray(np.asarray(W, dtype=np.float32)).reshape(1, F),
        "b": np.ascontiguousarray(np.asarray(b, dtype=np.float32)).reshape(1, 1),
    }
    # The device pool occasionally throws a transient
    # NRT_EXEC_UNIT_UNRECOVERABLE on dispatch; a retry lands cleanly.
    last_err = None
    for attempt in range(3):
        try:
            return run_bass_kernel_spmd(
                nc,
                [dict(in_map) for _ in range(NCORES)],
                core_ids=list(range(NCORES)),
                **run_kwargs,
            )
        except Exception as e:  # noqa: BLE001
            last_err = e
            import time
            time.sleep(2.0 * (attempt + 1))
            try:
                import jax
                jax.clear_caches()
                jax.clear_backends()
            except Exception:
                pass
    raise last_err


def kernel(x, W, b):
    res = _run(x, W, b)
    outs = res.results
    full = np.empty((T, 1 + NREP * T), dtype=np.float32)
    full[:, 0:1] = outs[0]["scores"]
    for c in range(NCORES):
        full[:, 1 + c * RPC * T: 1 + (c + 1) * RPC * T] = outs[c]["rep"]
    return full
